# revision 21
# baseline (speedup 1.0000x reference)
"""Trainium2 Bass kernel for nn_MeanDegConv (gnn_message_passing) on 8 NeuronCores.

Self-contained: imports the Bass/Tile stack from /opt/trn_rl_repo (part of the
container environment) and hardcodes all shapes/sharding for the problem.

Design: fold the MLPs algebraically so on-device work is two gather+segment-sum
stages (edge means, then vertex means) plus small dense matmuls.
  stage1: S[e] = sum_{i: edges[i]=e} X[vertex[i]];  xe = (S@K1)/deg_e + logdeg*k2 + c1
  allgather xe across cores (edge-sharded -> replicated)
  stage2: Z[v] = sum_{i: vertex[i]=v} xe[edges[i]];  out = relu(Z/deg_v + X@MX + X0@MX0
          + logdeg_v*r4 + c0) @ W3w2 + b2
Segment sums run as one-hot matmuls over gathered rows (gather via SWDGE
dma_gather, descriptors spread over 4 SWDGE queues = 4 Q7 core pairs).
SWDGE descriptor *generation* (~8ns/idx per queue pair) is the kernel
bottleneck, so sorted streams are greedily pair-merged: one 512B descriptor
fetches two consecutive table rows (overlapping pair tables XtabP/xe_allP),
cutting descriptor count ~17%.
All gathered data, one-hots and big matmuls are bf16 (psum accumulation f32).
"""
import sys
for _p in ('/opt/trn_rl_repo',):
    if _p not in sys.path:
        sys.path.insert(0, _p)

import numpy as np
import ml_dtypes

import concourse.bass as bass
import concourse.mybir as mybir
import concourse.tile as tile
import concourse.bacc as bacc
from concourse.bass_utils import run_bass_kernel_spmd

N, E, NNZ, D = 50000, 10000, 1000000, 128
C = 8
EPC, VPC = E // C, N // C          # 1250 edges, 6250 vertices per core
NWIN_E = (EPC + 127) // 128        # 10
NWIN_V = (VPC + 127) // 128        # 49
EP = NWIN_E * 128                  # 1280 padded edge slots per core
VP = NWIN_V * 128                  # 6272 padded vertex slots per core
CHUNK = 2048                       # single-row gather indices per dma_gather
                                   # (129 descs/engine fits the SWDGE ring)
CHUNKP = 1024                      # pair gather indices per call (same bytes)
SPLIT = 32768                      # int16 index limit for the X table
WIDE = 4                           # one-hot tiles built per DVE op
XE_ROWS = NWIN_E * C * 128         # 10240 rows in the gathered xe table

F32 = mybir.dt.float32
BF16 = mybir.dt.bfloat16
I16 = mybir.dt.int16
BF = ml_dtypes.bfloat16

NUM_Q = 4                          # SWDGE queues (4 Q7 core pairs)


def _pack_idx16(idx32: np.ndarray) -> np.ndarray:
    """[L] int32 -> [128, L/16] int16 in the dma_gather wrap layout."""
    L = len(idx32)
    assert L % 16 == 0
    a = idx32.astype(np.int16).reshape(L // 16, 16).T  # [16, L/16]
    return np.ascontiguousarray(np.tile(a, (8, 1)))    # [128, L/16]


def _pad_to(arr, L, fill):
    out = np.full(L, fill, arr.dtype)
    out[:len(arr)] = arr
    return out


def _pair_walk(rows, lids):
    """Greedy pair-merge of a row-sorted stream: adjacent entries whose rows
    are r, r+1 merge into one pair descriptor. Returns (pi, pla, plb, si, sl)."""
    pi, pla, plb, si, sl = [], [], [], [], []
    i, n = 0, len(rows)
    while i < n:
        if i + 1 < n and rows[i + 1] == rows[i] + 1:
            pi.append(rows[i]); pla.append(lids[i]); plb.append(lids[i + 1])
            i += 2
        else:
            si.append(rows[i]); sl.append(lids[i])
            i += 1
    return (np.asarray(pi, np.int32), np.asarray(pla, np.float32),
            np.asarray(plb, np.float32), np.asarray(si, np.int32),
            np.asarray(sl, np.float32))


def _build_single(per_win, tiles_per_win, chunk=CHUNK):
    """per_win: list of (idx, lid) arrays. Pad each window to T*128, concat,
    pad to chunk multiple. Returns idx [L], lid [L]."""
    idx_parts, lid_parts = [], []
    for (ix, ld), T in zip(per_win, tiles_per_win):
        L = T * 128
        idx_parts.append(_pad_to(ix, L, 0))
        lid_parts.append(_pad_to(ld, L, -1.0))
    idx = (np.concatenate(idx_parts) if idx_parts else np.zeros(0, np.int32))
    lid = (np.concatenate(lid_parts) if lid_parts else np.zeros(0, np.float32))
    Lt = ((len(idx) + chunk - 1) // chunk) * chunk
    return _pad_to(idx, Lt, 0), _pad_to(lid, Lt, -1.0)


def _build_pair(per_win, tiles_per_win, chunk=CHUNKP):
    """per_win: list of (idx, lida, lidb). Pads each window to T*128 pairs.
    Returns idx [L], lid [2L] (interleaved a/b column blocks of 128)."""
    idx_parts, lid_parts = [], []
    for (ix, la, lb), T in zip(per_win, tiles_per_win):
        L = T * 128
        idx_parts.append(_pad_to(ix, L, 0))
        a = _pad_to(la, L, -1.0).reshape(-1, 128)
        b = _pad_to(lb, L, -1.0).reshape(-1, 128)
        lid_parts.append(np.stack([a, b], axis=1).reshape(-1, 128))
    idx = (np.concatenate(idx_parts) if idx_parts else np.zeros(0, np.int32))
    lid = (np.concatenate(lid_parts) if lid_parts
           else np.zeros((0, 128), np.float32))
    Lt = ((len(idx) + chunk - 1) // chunk) * chunk
    idx = _pad_to(idx, Lt, 0)
    lid2 = np.full((2 * Lt // 128, 128), -1.0, np.float32)
    lid2[:len(lid)] = lid
    return idx, lid2


def _lid_cols(lid_flat):
    """[L] -> [128, L/128] bf16 (column t = lids of tile t)."""
    return np.ascontiguousarray(lid_flat.reshape(-1, 128).T.astype(BF))


def prepare(inputs):
    """Host-side preprocessing: consts, per-core streams, schedule."""
    X = np.asarray(inputs["X"], np.float32)
    X0 = np.asarray(inputs["X0"], np.float32)
    v = np.asarray(inputs["vertex"]).astype(np.int64)
    e = np.asarray(inputs["edges"]).astype(np.int64)
    W1_w = np.asarray(inputs["W1_w"], np.float32); W1_b = np.asarray(inputs["W1_b"], np.float32)
    W2_w = np.asarray(inputs["W2_w"], np.float32); W2_b = np.asarray(inputs["W2_b"], np.float32)
    W3_w1 = np.asarray(inputs["W3_w1"], np.float32); W3_b1 = np.asarray(inputs["W3_b1"], np.float32)
    W3_w2 = np.asarray(inputs["W3_w2"], np.float32); W3_b2 = np.asarray(inputs["W3_b2"], np.float32)

    deg_e = np.bincount(e, minlength=E).astype(np.float32)
    deg_v = np.bincount(v, minlength=N).astype(np.float32)

    # ---- folded weight matrices (float64 for accuracy, cast at the end)
    W2a = W2_w[:D].astype(np.float64); W2b1 = W2_w[D:2*D].astype(np.float64)
    w2b_log = W2_w[2*D].astype(np.float64)
    R1 = W3_w1[:D].astype(np.float64); R2 = W3_w1[D:2*D].astype(np.float64)
    R3 = W3_w1[2*D:3*D].astype(np.float64); r4 = W3_w1[3*D].astype(np.float64)
    W2bR = W2b1 @ R1
    K1 = (W1_w.astype(np.float64) @ W2bR).astype(np.float32)
    k2 = (w2b_log @ R1).astype(np.float32)
    c1 = (W1_b.astype(np.float64) @ W2bR).astype(np.float32)
    MX = (W2a @ R1 + R2).astype(np.float32)
    MX0 = R3.astype(np.float32)
    c0 = (W2_b.astype(np.float64) @ R1 + W3_b1).astype(np.float32)

    Xbf = X.astype(BF)
    XtabP = np.concatenate([Xbf[:SPLIT], np.vstack([Xbf[1:SPLIT + 1]])], axis=1)
    XtabBP = np.concatenate(
        [Xbf[SPLIT:], np.vstack([Xbf[SPLIT + 1:], np.zeros((1, D), BF)])],
        axis=1)

    iota = np.tile(np.arange(128, dtype=np.float32), (128, 1))
    iota4 = np.tile(np.arange(128, dtype=np.float32), (128, WIDE, 1))
    consts = {
        "iota": np.ascontiguousarray(iota.astype(BF)),
        "iota4": np.ascontiguousarray(iota4.astype(BF)),
        "K1": K1,
        "K2": np.ascontiguousarray(np.stack([k2, c1])),            # [2,128]
        "MX": np.ascontiguousarray(MX.astype(BF)),
        "MX0": np.ascontiguousarray(MX0.astype(BF)),
        "RC2": np.ascontiguousarray(np.stack([r4.astype(np.float32), c0]).astype(BF)),  # [2,128]
        "W3w2": W3_w2,
        "b2row": W3_b2.reshape(1, D),
        "ones1": np.ones((1, 128), np.float32),
        "Xtab": np.ascontiguousarray(Xbf),                         # bf16 table
        "XtabP": np.ascontiguousarray(XtabP),                      # [SPLIT, 256]
        "XtabBP": np.ascontiguousarray(XtabBP),                    # [N-SPLIT, 256]
    }

    # ---- stage-1: per (core, window, half) incidence lists
    core1 = (e // EPC).astype(np.int64)          # owning core by edge range
    win1 = ((e % EPC) // 128).astype(np.int64)   # window within core
    lidx1 = ((e % EPC) % 128).astype(np.float32) # slot within window
    half1 = (v >= SPLIT).astype(np.int64)

    key1 = (core1 * NWIN_E + win1) * 2 + half1
    order1 = np.argsort(key1, kind="stable")
    ks = key1[order1]
    bounds1 = np.searchsorted(ks, np.arange(C * NWIN_E * 2 + 1))

    def seg1(c, w, h):
        b = (c * NWIN_E + w) * 2 + h
        s = order1[bounds1[b]:bounds1[b + 1]]
        return s[np.argsort(v[s], kind="stable")]   # ascending table rows

    # pair-walk every (core, window, half) segment
    s1 = {}   # (c, w, h) -> (pi, pla, plb, si, sl)
    for c in range(C):
        for w in range(NWIN_E):
            for h in (0, 1):
                s = seg1(c, w, h)
                rows = (v[s] - (SPLIT if h else 0)).astype(np.int64)
                s1[(c, w, h)] = _pair_walk(rows, lidx1[s])

    def tmax(d, kidx, wr):
        return [int(np.ceil(max(len(d[(c, w)][kidx]) for c in range(C)) / 128))
                for w in range(wr)]

    sA = {(c, w): s1[(c, w, 0)] for c in range(C) for w in range(NWIN_E)}
    sB = {(c, w): s1[(c, w, 1)] for c in range(C) for w in range(NWIN_E)}
    TPA = tmax(sA, 0, NWIN_E); TSA = tmax(sA, 3, NWIN_E)
    TPB = tmax(sB, 0, NWIN_E); TSB = tmax(sB, 3, NWIN_E)

    # ---- stage-2: per (core, window) lists over the gathered xe table
    core2 = (v // VPC).astype(np.int64)
    win2 = ((v % VPC) // 128).astype(np.int64)
    lidx2 = ((v % VPC) % 128).astype(np.float32)
    # xe_all layout from two grouped allgathers: group1 = windows 0..7
    # ([core][win][slot]), group2 = windows 8..9 at offset 8*C*128
    w1 = (e % EPC) // 128
    co = e // EPC
    sl = (e % EPC) % 128
    rowid2 = np.where(
        w1 < 8,
        co * (8 * 128) + w1 * 128 + sl,
        8 * C * 128 + co * (2 * 128) + (w1 - 8) * 128 + sl)

    key2 = core2 * NWIN_V + win2
    order2 = np.argsort(key2, kind="stable")
    ks2 = key2[order2]
    bounds2 = np.searchsorted(ks2, np.arange(C * NWIN_V + 1))

    s2 = {}
    for c in range(C):
        for w in range(NWIN_V):
            b = c * NWIN_V + w
            s = order2[bounds2[b]:bounds2[b + 1]]
            s = s[np.argsort(rowid2[s], kind="stable")]
            s2[(c, w)] = _pair_walk(rowid2[s], lidx2[s])
    TP2 = tmax(s2, 0, NWIN_V); TS2 = tmax(s2, 3, NWIN_V)

    sched = {"TPA": TPA, "TSA": TSA, "TPB": TPB, "TSB": TSB,
             "TP2": TP2, "TS2": TS2}

    # ---- per-core input maps
    in_maps = []
    log_deg_e = np.log(deg_e); log_deg_v = np.log(deg_v)
    for c in range(C):
        def packs(d, T, wr):
            per = [(d[(c, w)][3], d[(c, w)][4]) for w in range(wr)]
            idx, lid = _build_single(per, T)
            return _pack_idx16(idx), _lid_cols(lid)

        def packp(d, T, wr):
            per = [(d[(c, w)][0], d[(c, w)][1], d[(c, w)][2]) for w in range(wr)]
            idx, lid2 = _build_pair(per, T)
            return _pack_idx16(idx), np.ascontiguousarray(lid2.T.astype(BF))

        iAs, lAs = packs(sA, TSA, NWIN_E)
        iAp, lAp = packp(sA, TPA, NWIN_E)
        iBs, lBs = packs(sB, TSB, NWIN_E)
        iBp, lBp = packp(sB, TPB, NWIN_E)
        i2s, l2s = packs(s2, TS2, NWIN_V)
        i2p, l2p = packp(s2, TP2, NWIN_V)

        # per-core edge aux (padded slots get deg=1, log=0)
        de = np.ones(EP, np.float32); de[:EPC] = deg_e[c*EPC:(c+1)*EPC]
        le = np.zeros(EP, np.float32); le[:EPC] = log_deg_e[c*EPC:(c+1)*EPC]
        auxe = np.ascontiguousarray(np.stack([de * le, de]))        # [2, EP]
        invdeg_e_col = np.ascontiguousarray(
            (1.0 / de).reshape(NWIN_E, 128).T)                      # [128, NWIN_E]

        dv = np.ones(VP, np.float32); dv[:VPC] = deg_v[c*VPC:(c+1)*VPC]
        lv = np.zeros(VP, np.float32); lv[:VPC] = log_deg_v[c*VPC:(c+1)*VPC]
        auxv = np.ascontiguousarray(np.stack([lv, np.ones(VP, np.float32)]).astype(BF))  # [2, VP]
        invdeg_bc = np.ascontiguousarray(
            np.tile(1.0 / dv, (128, 1)))                            # [128, VP]

        Xp = np.zeros((VP, D), np.float32); Xp[:VPC] = X[c*VPC:(c+1)*VPC]
        X0p = np.zeros((VP, D), np.float32); X0p[:VPC] = X0[c*VPC:(c+1)*VPC]

        m = dict(consts)
        m.update({
            "idxAs": iAs, "lidAs": lAs, "idxAp": iAp, "lidAp": lAp,
            "idxBs": iBs, "lidBs": lBs, "idxBp": iBp, "lidBp": lBp,
            "idx2s": i2s, "lid2s": l2s, "idx2p": i2p, "lid2p": l2p,
            "auxe": auxe, "invdeg_e_col": invdeg_e_col,
            "auxv": auxv, "invdeg_bc": invdeg_bc,
            "XT": np.ascontiguousarray(Xp.T.astype(BF)),
            "X0T": np.ascontiguousarray(X0p.T.astype(BF)),
        })
        in_maps.append(m)
    return in_maps, sched


def build(in_map0, sched, mode="full"):
    """Build the SPMD Bass program. in_map0 supplies shapes."""
    TPA, TSA = sched["TPA"], sched["TSA"]
    TPB, TSB = sched["TPB"], sched["TSB"]
    TP2, TS2 = sched["TP2"], sched["TS2"]
    nc = bacc.Bacc(None, num_swdge_queues=NUM_Q)

    def param(name, dt=F32):
        arr = in_map0[name]
        return nc.declare_dram_parameter(name, list(arr.shape), dt, isOutput=False)

    Xtab_d = param("Xtab", BF16)
    XtabP_d = param("XtabP", BF16)
    XtabBP_d = param("XtabBP", BF16)
    iota_d = param("iota", BF16); iota4_d = param("iota4", BF16)
    K1_d = param("K1"); K2_d = param("K2")
    MX_d = param("MX", BF16); MX0_d = param("MX0", BF16); RC2_d = param("RC2", BF16)
    W3w2_d = param("W3w2"); b2row_d = param("b2row"); ones1_d = param("ones1")
    sparams = {}
    for nm in ("idxAs", "idxAp", "idxBs", "idxBp", "idx2s", "idx2p"):
        sparams[nm] = param(nm, I16)
    for nm in ("lidAs", "lidAp", "lidBs", "lidBp", "lid2s", "lid2p"):
        sparams[nm] = param(nm, BF16)
    auxe_d = param("auxe"); invde_d = param("invdeg_e_col")
    auxv_d = param("auxv", BF16); invbc_d = param("invdeg_bc")
    XT_d = param("XT", BF16); X0T_d = param("X0T", BF16)
    out_d = nc.declare_dram_parameter("out", [VP, D], F32, isOutput=True)

    def nch(nm, chunk):
        return in_map0[nm].shape[1] * 16 // chunk

    qctr = [1]   # start at 1: queue-0 gathers block the Pool engine for the
                 # full descriptor-gen; queues 1-3 dispatch in ~70ns

    def next_q():
        q = qctr[0] % NUM_Q
        qctr[0] += 1
        return q

    from contextlib import ExitStack
    with tile.TileContext(nc) as tc:
        with ExitStack() as _es:
            def _pool(**kw):
                return _es.enter_context(tc.tile_pool(**kw))
            cp = _pool(name="const", bufs=1)
            sp = _pool(name="stream", bufs=1)
            gAsp = _pool(name="gAs", bufs=4)
            gApp = _pool(name="gAp", bufs=3)
            gBsp = _pool(name="gBs", bufs=3)
            gBpp = _pool(name="gBp", bufs=2)
            g2sp = _pool(name="g2s", bufs=4)
            g2pp = _pool(name="g2p", bufs=3)
            pwp = _pool(name="pw", bufs=8)
            wp = _pool(name="work", bufs=3)
            fwp = _pool(name="fw", bufs=4)
            accp = _pool(name="acc", bufs=1)
            psS = _pool(name="psS", bufs=1, space="PSUM")
            psXE = _pool(name="psXE", bufs=1, space="PSUM")
            psT = _pool(name="psT", bufs=1, space="PSUM")
            psR = _pool(name="psR", bufs=2, space="PSUM")
            psO = _pool(name="psO", bufs=1, space="PSUM")
            dp = _pool(name="dram", bufs=1, space="DRAM")
            # ---- load constants / streams
            def load(pool, dram_ap, name, dt=F32, eng=None):
                t = pool.tile(list(dram_ap.shape), dt, name=name, tag=name)
                (eng or nc.sync).dma_start(t[:], dram_ap[:])
                return t

            st = {}
            for nm in ("idxAs", "idxAp", "idxBs", "idxBp", "idx2s", "idx2p"):
                st[nm] = load(sp, sparams[nm], nm, I16)
            for nm in ("lidAs", "lidAp", "lidBs", "lidBp", "lid2s", "lid2p"):
                st[nm] = load(sp, sparams[nm], nm, BF16)
            iota_t = load(cp, iota_d, "iota", BF16)
            iota4_t = load(cp, iota4_d, "iota4", BF16)
            K1_t = load(cp, K1_d, "K1"); K2_t = load(cp, K2_d, "K2")
            MX_t = load(cp, MX_d, "MX", BF16); MX0_t = load(cp, MX0_d, "MX0", BF16)
            RC2_t = load(cp, RC2_d, "RC2", BF16)
            W3w2_t = load(cp, W3w2_d, "W3w2"); b2row_t = load(cp, b2row_d, "b2row")
            ones1_t = load(cp, ones1_d, "ones1")
            auxe_t = load(cp, auxe_d, "auxe"); invde_t = load(cp, invde_d, "invde")
            auxv_t = load(cp, auxv_d, "auxv", BF16)
            xe_local = dp.tile([EP, D], BF16)
            xe_g1 = dp.tile([C * 8 * 128, D], BF16, addr_space="Shared",
                            name="xe_g1")
            xe_g2 = dp.tile([C * 2 * 128, D], BF16, addr_space="Shared",
                            name="xe_g2")
            # overlapping pair table: row i = (xe_all[i], xe_all[i+1])
            xe_allP = dp.tile([XE_ROWS, 2 * D], BF16)

            sA_sb = accp.tile([128, EP], F32)   # S^T accumulated

            # ================= stage 1 =================
            PRE = 3   # chunks to prefetch ahead of consumption

            class Stream:
                def __init__(self, pool, tag, idx_t, lid_t, in_ap, nch,
                             sub=1, chunk=CHUNK, pre=PRE, elem_step=None):
                    self.pool, self.tag = pool, tag
                    self.idx_t, self.lid_t = idx_t, lid_t
                    self.in_ap, self.nch = in_ap, nch
                    self.sub, self.chunk, self.pre = sub, chunk, pre
                    self.elem_step = elem_step
                    self.spc = (chunk // 128) * sub  # subtiles per chunk
                    self.chunks = {}
                    self.next_issue = 0
                    self.tc = 0
                    self.pws = {}

                def _issue(self):
                    ci = self.next_issue
                    g = self.pool.tile([128, self.chunk // 128, self.sub * 128],
                                       BF16, tag=self.tag,
                                       name=f"{self.tag}{ci}")
                    nc.gpsimd.dma_gather(
                        out_ap=g[:], in_ap=self.in_ap,
                        idxs_ap=self.idx_t[:, ci * (self.chunk // 16):
                                           (ci + 1) * (self.chunk // 16)],
                        num_idxs=self.chunk, num_idxs_reg=self.chunk,
                        single_packet=False, elem_size=self.sub * D,
                        elem_step=self.elem_step, queue_num=next_q())
                    self.chunks[ci] = g
                    self.next_issue += 1

                def tile(self):
                    ci = self.tc // self.spc
                    while self.next_issue <= min(ci + self.pre, self.nch - 1):
                        self._issue()
                    within = self.tc % self.spc
                    pt, sb = within // self.sub, within % self.sub
                    g = self.chunks[ci][:, pt, sb * 128:(sb + 1) * 128]
                    grp = self.tc // WIDE
                    if grp not in self.pws:
                        base = grp * WIDE
                        nwide = min(WIDE, self.lid_t.shape[1] - base)
                        pw = pwp.tile([128, WIDE, 128], BF16, tag="pw",
                                      name=f"pw{self.tag}{grp}")
                        lcols = self.lid_t[:, base:base + nwide]
                        nc.vector.tensor_tensor(
                            out=pw[:, :nwide, :],
                            in0=iota4_t[:, :nwide, :],
                            in1=lcols.unsqueeze(2).broadcast_to([128, nwide, 128]),
                            op=mybir.AluOpType.is_equal)
                        self.pws[grp] = pw
                    p = self.pws[grp][:, (self.tc % WIDE), :]
                    self.tc += 1
                    return g, p

            sAs = Stream(gAsp, "gAs", st["idxAs"], st["lidAs"],
                         Xtab_d[0:SPLIT, :], nch("idxAs", CHUNK))
            sAp = Stream(gApp, "gAp", st["idxAp"], st["lidAp"],
                         XtabP_d[:], nch("idxAp", CHUNKP), sub=2, chunk=CHUNKP)
            sBs = Stream(gBsp, "gBs", st["idxBs"], st["lidBs"],
                         Xtab_d[SPLIT:N, :], nch("idxBs", CHUNK))
            sBp = Stream(gBpp, "gBp", st["idxBp"], st["lidBp"],
                         XtabBP_d[:], nch("idxBp", CHUNKP), sub=2, chunk=CHUNKP)

            for w in range(NWIN_E):
                T = 2 * TPA[w] + TSA[w] + 2 * TPB[w] + TSB[w]
                ps0 = psS.tile([128, 128], F32, tag="s1a", name=f"ps0w{w}")
                ps1 = (psS.tile([128, 128], F32, tag="s1b", name=f"ps1w{w}")
                       if T > 1 else None)
                pp = [ps0, ps1]
                j = 0
                for strm, cnt in ((sAp, 2 * TPA[w]), (sAs, TSA[w]),
                                  (sBp, 2 * TPB[w]), (sBs, TSB[w])):
                    for _ in range(cnt):
                        g, p = strm.tile()
                        nc.tensor.matmul(pp[j % 2][:], g, p,
                                         start=(j < 2), stop=(j >= T - 2))
                        j += 1
                sl = sA_sb[:, w * 128:(w + 1) * 128]
                nc.scalar.copy(sl, ps0[:])
                if T > 1:
                    nc.vector.tensor_tensor(out=sl, in0=sl, in1=ps1[:],
                                            op=mybir.AluOpType.add)

                # xe_hat for this window: psum = S^T.T@K1 + auxe.T@K2, /deg
                ps = psXE.tile([128, 128], F32, tag="xe")
                nc.tensor.matmul(ps[:], sA_sb[:, w * 128:(w + 1) * 128], K1_t[:],
                                 start=True, stop=False)
                nc.tensor.matmul(ps[:], auxe_t[:, w * 128:(w + 1) * 128],
                                 K2_t[:], start=False, stop=True)
                xe_sb = wp.tile([128, D], BF16, tag="xe_sb")
                nc.scalar.activation(
                    out=xe_sb[:], in_=ps[:],
                    func=mybir.ActivationFunctionType.Copy,
                    scale=invde_t[:, w:w + 1])
                nc.sync.dma_start(xe_local[w * 128:(w + 1) * 128, :], xe_sb[:])
                if mode in ("s1ag", "full"):
                    if w == 7:
                        nc.gpsimd.collective_compute(
                            "AllGather", mybir.AluOpType.bypass,
                            replica_groups=[list(range(C))],
                            ins=[xe_local[0:8 * 128, :].opt()],
                            outs=[xe_g1.opt()])
                        # xe_allP rows 0..8190 fully determined by group 1
                        nc.sync.dma_start(
                            xe_allP[0:8 * C * 128, 0:D], xe_g1[:])
                        nc.sync.dma_start(
                            xe_allP[0:8 * C * 128 - 1, D:2 * D],
                            xe_g1[1:8 * C * 128, :])
                    elif w == NWIN_E - 1:
                        nc.gpsimd.collective_compute(
                            "AllGather", mybir.AluOpType.bypass,
                            replica_groups=[list(range(C))],
                            ins=[xe_local[8 * 128:10 * 128, :].opt()],
                            outs=[xe_g2.opt()])
                        g1n = 8 * C * 128
                        nc.sync.dma_start(
                            xe_allP[g1n:XE_ROWS, 0:D], xe_g2[:])
                        nc.sync.dma_start(
                            xe_allP[g1n - 1:g1n, D:2 * D], xe_g2[0:1, :])
                        nc.sync.dma_start(
                            xe_allP[g1n:XE_ROWS - 1, D:2 * D],
                            xe_g2[1:2 * C * 128, :])

            if mode == "s1":
                for w in range(NWIN_E):
                    xe_rd = wp.tile([128, D], BF16, tag="xe_rd", name="xe_rd")
                    nc.sync.dma_start(xe_rd[:], xe_local[w * 128:(w + 1) * 128, :])
                    o32 = wp.tile([128, D], F32, tag="o32", name="o32")
                    nc.vector.tensor_scalar(
                        out=o32[:], in0=xe_rd[:], scalar1=0.0, scalar2=None,
                        op0=mybir.AluOpType.add)
                    nc.sync.dma_start(out_d[w * 128:(w + 1) * 128, :], o32[:])
            if mode == "full":
                # ================= stage 2 =================
                def finish_window(w, ps0, ps1):
                    sl = slice(w * 128, (w + 1) * 128)
                    xt = fwp.tile([128, 128], BF16, tag="xt", name="xt")
                    x0t = fwp.tile([128, 128], BF16, tag="x0t", name="x0t")
                    invbc = fwp.tile([128, 128], F32, tag="invbc", name="invbc")
                    nc.sync.dma_start(xt[:], XT_d[:, sl])
                    nc.sync.dma_start(x0t[:], X0T_d[:, sl])
                    nc.sync.dma_start(invbc[:], invbc_d[:, sl])
                    psr = psR.tile([128, 128], F32, tag="r", name="psr")
                    nc.tensor.matmul(psr[:], MX_t[:], xt[:], start=True, stop=False)
                    nc.tensor.matmul(psr[:], MX0_t[:], x0t[:], start=False, stop=False)
                    nc.tensor.matmul(psr[:], RC2_t[:], auxv_t[:, sl], start=False, stop=True)
                    pre = wp.tile([128, 128], F32, tag="pre", name="pre")
                    nc.vector.tensor_tensor(out=pre[:], in0=ps0[:], in1=invbc[:],
                                            op=mybir.AluOpType.mult)
                    if ps1 is not None:
                        tmp = wp.tile([128, 128], F32, tag="tmp", name="tmp")
                        nc.vector.tensor_tensor(out=tmp[:], in0=ps1[:], in1=invbc[:],
                                                op=mybir.AluOpType.mult)
                        nc.vector.tensor_tensor(out=pre[:], in0=pre[:], in1=tmp[:],
                                                op=mybir.AluOpType.add)
                    nc.vector.tensor_tensor(out=pre[:], in0=pre[:], in1=psr[:],
                                            op=mybir.AluOpType.add)
                    relu = wp.tile([128, 128], F32, tag="relu", name="relu")
                    nc.scalar.activation(out=relu[:], in_=pre[:],
                                         func=mybir.ActivationFunctionType.Relu)
                    pso = psO.tile([128, 128], F32, tag="o", name="pso")
                    nc.tensor.matmul(pso[:], relu[:], W3w2_t[:], start=True, stop=False)
                    nc.tensor.matmul(pso[:], ones1_t[:], b2row_t[:], start=False, stop=True)
                    o_sb = wp.tile([128, D], F32, tag="o_sb", name="o_sb")
                    nc.scalar.copy(o_sb[:], pso[:])
                    nc.sync.dma_start(out_d[w * 128:(w + 1) * 128, :], o_sb[:])

                s2s = Stream(g2sp, "g2s", st["idx2s"], st["lid2s"],
                             xe_allP[:, 0:D], nch("idx2s", CHUNK),
                             elem_step=2 * D)
                s2p = Stream(g2pp, "g2p", st["idx2p"], st["lid2p"],
                             xe_allP[:], nch("idx2p", CHUNKP),
                             sub=2, chunk=CHUNKP)
                def do_window2(w):
                    T = 2 * TP2[w] + TS2[w]
                    ps0 = psT.tile([128, 128], F32, tag="t3a", name=f"psT0w{w}")
                    ps1 = (psT.tile([128, 128], F32, tag="t3b", name=f"psT1w{w}")
                           if T > 1 else None)
                    pp = [ps0, ps1]
                    j = 0
                    for strm, cnt in ((s2p, 2 * TP2[w]), (s2s, TS2[w])):
                        for _ in range(cnt):
                            g, p = strm.tile()
                            nc.tensor.matmul(pp[j % 2][:], g, p,
                                             start=(j < 2), stop=(j >= T - 2))
                            j += 1
                    finish_window(w, ps0, ps1)

                for w in range(NWIN_V):
                    do_window2(w)

    nc.finalize()
    return nc


def run(trace=False, mode="full", **inputs):
    in_maps, sched = prepare(inputs)
    nc = build(in_maps[0], sched, mode=mode)
    res = run_bass_kernel_spmd(nc, in_maps, list(range(C)), trace=trace)
    out = np.concatenate([res.results[c]["out"][:VPC] for c in range(C)], axis=0)
    return out, res


def kernel(**inputs):
    """Harness entry point: full inputs in, full [N, D] float32 output."""
    out, _res = run(trace=False, mode="full", **inputs)
    return out.astype(np.float32)
